# revision 13
# baseline (speedup 1.0000x reference)
"""RWKV ChannelMixer (single-token) on 8 Trainium2 NeuronCores.

Math (reference):
    xn  = LayerNorm(x) * ln_w + ln_b
    xk  = xn*tmk + prev*(1-tmk);  xr = xn*tmr + prev*(1-tmr)
    r   = sigmoid(rw @ xr)                       # (D,)
    k   = relu(kw @ xk)^2                        # (F,)
    out = x + r * (vw @ k)                       # (D,)
    returns (out, xn)

Sharding (8 cores, no collectives):
    kw: F-row-sharded (512 rows/core)  -> local k chunk (512,)
    vw: F-col-sharded (512 cols/core)  -> partial v_i (1024,)
    rw: D-row-sharded (128 rw rows/core) -> pre-sigmoid r chunk (128,)
    Host computes the O(D) LN/token-mix prologue (0.01% of FLOPs) and
    ships the mixed vectors pre-transposed; host unshard: v = sum_i v_i,
    r = sigmoid(concat(r_i)), out = x + r*v, xn from the same LN.

Device: the memory-bound work -- streaming 2.25MB/core of fp16 weights
(rel err ~2e-4) -- runs as accumulating [1,N] Tensor-engine matmuls that
trail the DMA chunk stream (weights are the moving operand at 1 cycle/
row).  rw's mat-vec runs on the otherwise-idle Vector engine so PE goes
straight from kw into vw.  Junk fp32 matmuls up front keep the PE
p-state ramped.  All engine APs keep partition base 0 (BIR verifier
rejects unaligned/strided partition access).
"""

import sys
import numpy as np

for _p in ("/opt/trn_rl_repo", "/root/.axon_site/_ro/trn_rl_repo"):
    if _p not in sys.path:
        sys.path.append(_p)

D = 1024
F = 4096
N_CORES = 8
FSH = F // N_CORES      # 512 kw rows / vw cols per core
DSH = D // N_CORES      # 128 rw rows per core
LN_EPS = 1e-5

_STATE = {}


def _body(nc, tc, mybir, stage):
    f32 = mybir.dt.float32
    f16 = mybir.dt.float16
    Alu = mybir.AluOpType
    Act = mybir.ActivationFunctionType

    kw_d = nc.dram_tensor("kw_p", [128, 4096], f16, kind="ExternalInput").ap()
    vw_d = nc.dram_tensor("vw_p", [128, 4096], f16, kind="ExternalInput").ap()
    rw_d = nc.dram_tensor("rw_p", [128, 1024], f16, kind="ExternalInput").ap()
    # xkT (cols 0-7) and xrT (cols 8-15): host-mixed vectors, d-chunk major
    sm_d = nc.dram_tensor("smalls", [128, 16], f16, kind="ExternalInput").ap()

    v_d = nc.dram_tensor("v_out", [1, 1024], f32, kind="ExternalOutput").ap()
    r_d = nc.dram_tensor("r_out", [1, 128], f32, kind="ExternalOutput").ap()

    import contextlib
    with contextlib.ExitStack() as ctx:
        wp = ctx.enter_context(tc.tile_pool(name="w", bufs=1))
        vp = ctx.enter_context(tc.tile_pool(name="v", bufs=1))
        pp = ctx.enter_context(tc.tile_pool(name="ps", bufs=1, space="PSUM"))

        # ---- DMA: tiny smalls on the ACT HWDGE ring (parallel with the
        # SP ring), bulk weights on the SP ring in consumption order:
        # rw first (its mat-vec runs on DVE while PE chews kw), then kw,
        # then vw, chunked so compute trails the stream.
        sm_sb = vp.tile([128, 16], f16, tag="sm")
        nc.scalar.dma_start(out=sm_sb[:], in_=sm_d[:])
        xkT = sm_sb[:, 0:8]
        xrT = sm_sb[:, 8:16]

        kw_sb = wp.tile([128, 4096], f16, tag="kw")
        rw_sb = wp.tile([128, 1024], f16, tag="rw")
        vw_sb = wp.tile([128, 4096], f16, tag="vw")
        # 7 bulk DMAs total: stays within the 8 HWDGE semaphore lanes so
        # the issue pipeline never stalls on semaphore recycling
        CH = [(0, 2048), (2048, 3584), (3584, 4096)]
        if stage >= 2:
            nc.sync.dma_start(out=rw_sb[:], in_=rw_d[:])
            for a, b in CH:
                nc.sync.dma_start(out=kw_sb[:, a:b], in_=kw_d[:, a:b])
            for a, b in CH:
                nc.sync.dma_start(out=vw_sb[:, a:b], in_=vw_d[:, a:b])

        # ---- constants; dummy activation forces the single ACT table
        # load early (Relu/Copy share every table set)
        ones16 = vp.tile([128, 1], f16, tag="ones16")
        nc.vector.memset(ones16[:], 1.0)
        ones_c128 = vp.tile([128, 1], f32, tag="ones_c128")
        nc.vector.memset(ones_c128[:], 1.0)
        dummy = vp.tile([1, 1], f32, tag="dummy")
        nc.scalar.activation(dummy[:], ones_c128[0:1, 0:1], Act.Relu)
        junk = vp.tile([128, 512], f32, tag="junk")
        nc.vector.memset(junk[:], 1.0)
        from concourse.masks import make_identity
        ident = vp.tile([128, 128], f32, tag="ident")
        make_identity(nc, ident)

        # ---- PSUM tiles
        warm_ps = pp.tile([1, 512], f32, tag="warm", bufs=1)
        k_ps = pp.tile([1, 512], f32, tag="kps", bufs=1)
        kT_ps = pp.tile([128, 4], f32, tag="kT", bufs=1)
        r_ps = pp.tile([1, 128], f32, tag="rps", bufs=1)
        v0_ps = pp.tile([1, 512], f32, tag="v0", bufs=1)
        v1_ps = pp.tile([1, 512], f32, tag="v1", bufs=1)

        # ---- PE p-state warm-up: big junk fp32 matmuls (~2048 cycles
        # each) keep the PE continuously busy until the first kw chunk
        # lands, so the real matmuls run at the ramped clock
        for _ in range(4):
            nc.tensor.matmul(warm_ps[:], ones_c128[:], junk[:],
                             start=True, stop=True)

        if stage < 2:
            return

        # ---- r = rw @ xr on the DVE (fp16 accumulate; PE stays on the
        # big streams).  acc_r[p, j] = sum_c rw_sb[p, c*128+j]*xr[128c+p]
        # DVE scalar operands must be fp32 -- cast xrT once on ACT.
        xrT_f = vp.tile([128, 8], f32, tag="xrTf")
        nc.scalar.copy(xrT_f[:], xrT)
        acc_r = vp.tile([128, 128], f16, tag="accr")
        nc.vector.tensor_scalar_mul(acc_r[:], rw_sb[:, 0:128], xrT_f[:, 0:1])
        for c in range(1, 8):
            nc.vector.scalar_tensor_tensor(
                out=acc_r[:], in0=rw_sb[:, c * 128:(c + 1) * 128],
                scalar=xrT_f[:, c:c + 1], in1=acc_r[:],
                op0=Alu.mult, op1=Alu.add)

        # ---- k = kw @ xk: accumulating [1,512] matmuls per d-chunk c
        for c in range(8):
            nc.tensor.matmul(k_ps[:], xkT[:, c:c + 1],
                             kw_sb[:, c * 512:(c + 1) * 512],
                             start=(c == 0), stop=(c == 7))

        # cross-partition reduce of acc_r while the k epilogue drains
        nc.tensor.matmul(r_ps[:], ones16[:], acc_r[:], start=True, stop=True)

        # ---- k epilogue: relu halves in parallel on DVE+ACT, 4 tiny PE
        # transposes, then square+cast to the fp16 stationary columns
        krelu = vp.tile([1, 640], f32, tag="krelu")
        nc.vector.tensor_scalar_max(krelu[:, 0:256], k_ps[:, 0:256], 0.0)
        nc.scalar.activation(krelu[:, 256:512], k_ps[:, 256:512], Act.Relu)
        for m in range(4):
            nc.tensor.transpose(kT_ps[:, m:m + 1],
                                krelu[0:1, m * 128:(m + 1) * 128],
                                ident[0:1, 0:1])
        kT_f = vp.tile([128, 4], f32, tag="kTf")
        nc.scalar.copy(kT_f[:], kT_ps[:])
        kT_h = vp.tile([128, 4], f16, tag="kTh")
        nc.scalar.square(kT_h[:], kT_f[:])

        # r evacuation aliases a dead region of krelu: the tile-level dep
        # keeps ACT from hoisting it (and the 0.6us r DMA issue) into the
        # hinge ahead of the critical k path
        r_row = krelu[0:1, 512:640]
        nc.scalar.copy(r_row, r_ps[:])
        nc.scalar.dma_start(out=r_d[:], in_=r_row)

        if stage < 3:
            return

        # ---- v partial: two [1,512] banks (d-halves), accumulated over
        #      4 f-chunks c; v0's last matmul lands first so its DVE
        #      evacuation overlaps v1's last matmul
        for c in range(4):
            nc.tensor.matmul(v0_ps[:], kT_h[:, c:c + 1],
                             vw_sb[:, c * 1024: c * 1024 + 512],
                             start=(c == 0), stop=(c == 3))
            nc.tensor.matmul(v1_ps[:], kT_h[:, c:c + 1],
                             vw_sb[:, c * 1024 + 512: c * 1024 + 1024],
                             start=(c == 0), stop=(c == 3))

        v_sb = vp.tile([1, 1024], f32, tag="vsb")
        nc.vector.tensor_copy(v_sb[:, 0:512], v0_ps[:])
        nc.scalar.copy(v_sb[:, 512:1024], v1_ps[:])
        nc.scalar.dma_start(out=v_d[:], in_=v_sb[:])


def _build(stage=3):
    import concourse.bacc as bacc
    import concourse.tile as tile
    from concourse import mybir

    nc = bacc.Bacc("TRN2", target_bir_lowering=False, debug=False,
                   num_devices=N_CORES)
    with tile.TileContext(nc) as tc:
        _body(nc, tc, mybir, stage)
    nc.compile()
    return nc


def _prep_shared(kw, vw, rw):
    """Slice + pack weights per core as fp16 in matmul-moving layout."""
    kw_p, vw_p, rw_p = [], [], []
    for i in range(N_CORES):
        # kw shard (512f, 1024d): [f, c, p] -> [p, c*512+f]
        A = kw[i * FSH:(i + 1) * FSH, :].reshape(512, 8, 128)
        kw_p.append(A.transpose(2, 1, 0).reshape(128, 4096)
                    .astype(np.float16))
        # vw shard (1024d, 512f): [d, c, p] -> [p, c*1024+d]
        B = vw[:, i * FSH:(i + 1) * FSH].reshape(1024, 4, 128)
        vw_p.append(B.transpose(2, 1, 0).reshape(128, 4096)
                    .astype(np.float16))
        # rw shard (128dout, 1024d): [j, c, p] -> [p, c*128+j]
        C = rw[i * DSH:(i + 1) * DSH, :].reshape(128, 8, 128)
        rw_p.append(C.transpose(2, 1, 0).reshape(128, 1024)
                    .astype(np.float16))
    return kw_p, vw_p, rw_p


def _prep_smalls(x, state, tmk, tmr, lnw, lnb):
    """Host LN + token mix; returns [128, 16] fp16 (xkT | xrT)."""
    mu = float(x.mean())
    var = float(np.square(x - mu).mean())
    xn = (x - mu) / np.sqrt(var + LN_EPS) * lnw + lnb
    prev = state[0]
    xk = xn * tmk + prev * (1.0 - tmk)
    xr = xn * tmr + prev * (1.0 - tmr)
    sm = np.concatenate([xk.reshape(8, 128).T, xr.reshape(8, 128).T], axis=1)
    return np.ascontiguousarray(sm).astype(np.float16)


def kernel(x, state, time_mix_k, time_mix_r, kw, vw, rw, ln_weight, ln_bias):
    from concourse import bass_utils

    x = np.asarray(x, dtype=np.float32)
    state = np.asarray(state, dtype=np.float32)
    kw = np.asarray(kw, dtype=np.float32)
    vw = np.asarray(vw, dtype=np.float32)
    rw = np.asarray(rw, dtype=np.float32)
    tmk = np.asarray(time_mix_k, dtype=np.float32)
    tmr = np.asarray(time_mix_r, dtype=np.float32)
    lnw = np.asarray(ln_weight, dtype=np.float32)
    lnb = np.asarray(ln_bias, dtype=np.float32)

    if "nc" not in _STATE:
        _STATE["nc"] = _build()
    nc = _STATE["nc"]

    kw_p, vw_p, rw_p = _prep_shared(kw, vw, rw)
    sm = _prep_smalls(x, state, tmk, tmr, lnw, lnb)

    in_maps = [{"kw_p": kw_p[i], "vw_p": vw_p[i], "rw_p": rw_p[i], "smalls": sm}
               for i in range(N_CORES)]

    res = bass_utils.run_bass_kernel_spmd(nc, in_maps, core_ids=list(range(N_CORES)))

    # unshard: v = sum of partials, r = sigmoid(concat of chunks)
    v = np.zeros(D, dtype=np.float64)
    for i in range(N_CORES):
        v += res.results[i]["v_out"].reshape(D).astype(np.float64)
    r_pre = np.concatenate([res.results[i]["r_out"].reshape(DSH)
                            for i in range(N_CORES)]).astype(np.float64)
    r = 1.0 / (1.0 + np.exp(-r_pre))
    out = x + (r * v).astype(np.float32)

    # xn: exact fp32 LN on host (auxiliary state output)
    mu = float(x.mean())
    var = float(np.square(x - mu).mean())
    xn = (x - mu) / np.sqrt(var + LN_EPS) * lnw + lnb
    return np.asarray(out, dtype=np.float32), np.asarray(xn, dtype=np.float32)


# revision 15
# speedup vs baseline: 1.0635x; 1.0635x over previous
"""RWKV ChannelMixer (single-token) on 8 Trainium2 NeuronCores.

Math (reference):
    xn  = LayerNorm(x) * ln_w + ln_b
    xk  = xn*tmk + prev*(1-tmk);  xr = xn*tmr + prev*(1-tmr)
    r   = sigmoid(rw @ xr)                       # (D,)
    k   = relu(kw @ xk)^2                        # (F,)
    out = x + r * (vw @ k)                       # (D,)
    returns (out, xn)

Sharding (8 cores, no collectives):
    kw: F-row-sharded (512 rows/core)  -> local k chunk (512,)
    vw: F-col-sharded (512 cols/core)  -> partial v_i (1024,)
    rw: D-row-sharded (128 rw rows/core) -> pre-sigmoid r chunk (128,)
    Host computes the O(D) LN/token-mix prologue (0.01% of FLOPs) and
    ships the mixed vectors pre-transposed; host unshard: v = sum_i v_i,
    r = sigmoid(concat(r_i)), out = x + r*v, xn from the same LN.

Device: the memory-bound work -- streaming 2.25MB/core of fp16 weights
(rel err ~2e-4) -- runs as accumulating [1,N] Tensor-engine matmuls that
trail the DMA chunk stream (weights are the moving operand at 1 cycle/
row).  rw's mat-vec runs on the otherwise-idle Vector engine so PE goes
straight from kw into vw.  Junk fp32 matmuls up front keep the PE
p-state ramped.  All engine APs keep partition base 0 (BIR verifier
rejects unaligned/strided partition access).
"""

import sys
import numpy as np

for _p in ("/opt/trn_rl_repo", "/root/.axon_site/_ro/trn_rl_repo"):
    if _p not in sys.path:
        sys.path.append(_p)

D = 1024
F = 4096
N_CORES = 8
FSH = F // N_CORES      # 512 kw rows / vw cols per core
DSH = D // N_CORES      # 128 rw rows per core
LN_EPS = 1e-5

_STATE = {}


def _body(nc, tc, mybir, stage):
    f32 = mybir.dt.float32
    f16 = mybir.dt.float16
    Alu = mybir.AluOpType
    Act = mybir.ActivationFunctionType

    kw_d = nc.dram_tensor("kw_p", [128, 4096], f16, kind="ExternalInput").ap()
    vw_d = nc.dram_tensor("vw_p", [128, 4096], f16, kind="ExternalInput").ap()
    rw_d = nc.dram_tensor("rw_p", [128, 1024], f16, kind="ExternalInput").ap()
    # xkT (cols 0-7) and xrT (cols 8-15): host-mixed vectors, d-chunk major
    sm_d = nc.dram_tensor("smalls", [128, 16], f16, kind="ExternalInput").ap()

    v_d = nc.dram_tensor("v_out", [1, 1024], f32, kind="ExternalOutput").ap()
    r_d = nc.dram_tensor("r_out", [1, 128], f32, kind="ExternalOutput").ap()

    import contextlib
    with contextlib.ExitStack() as ctx:
        wp = ctx.enter_context(tc.tile_pool(name="w", bufs=1))
        vp = ctx.enter_context(tc.tile_pool(name="v", bufs=1))
        pp = ctx.enter_context(tc.tile_pool(name="ps", bufs=1, space="PSUM"))

        # ---- DMA: tiny smalls on the ACT HWDGE ring (parallel with the
        # SP ring), bulk weights on the SP ring in consumption order:
        # rw first (its mat-vec runs on DVE while PE chews kw), then kw,
        # then vw, chunked so compute trails the stream.
        sm_sb = vp.tile([128, 16], f16, tag="sm")
        nc.scalar.dma_start(out=sm_sb[:], in_=sm_d[:])
        xkT = sm_sb[:, 0:8]
        xrT = sm_sb[:, 8:16]

        kw_sb = wp.tile([128, 4096], f16, tag="kw")
        rw_sb = wp.tile([128, 1024], f16, tag="rw")
        vw_sb = wp.tile([128, 4096], f16, tag="vw")
        # 7 bulk DMAs total: stays within the 8 HWDGE semaphore lanes so
        # the issue pipeline never stalls on semaphore recycling
        CH = [(0, 2048), (2048, 3584), (3584, 4096)]
        if stage >= 2:
            nc.sync.dma_start(out=rw_sb[:], in_=rw_d[:])
            for a, b in CH:
                nc.sync.dma_start(out=kw_sb[:, a:b], in_=kw_d[:, a:b])
            for a, b in CH:
                nc.sync.dma_start(out=vw_sb[:, a:b], in_=vw_d[:, a:b])

        # ---- constants; dummy activation forces the single ACT table
        # load early (Relu/Copy share every table set)
        ones16 = vp.tile([128, 1], f16, tag="ones16")
        nc.vector.memset(ones16[:], 1.0)
        ones_c128 = vp.tile([128, 1], f32, tag="ones_c128")
        nc.vector.memset(ones_c128[:], 1.0)
        dummy = vp.tile([1, 1], f32, tag="dummy")
        nc.scalar.activation(dummy[:], ones_c128[0:1, 0:1], Act.Relu)
        junk = vp.tile([128, 512], f32, tag="junk")
        nc.vector.memset(junk[:], 1.0)
        from concourse.masks import make_identity
        ident = vp.tile([128, 128], f32, tag="ident")
        make_identity(nc, ident)

        # ---- PSUM tiles
        warm_ps = pp.tile([1, 512], f32, tag="warm", bufs=1)
        k_ps = pp.tile([1, 512], f32, tag="kps", bufs=1)
        kT_ps = pp.tile([128, 4], f32, tag="kT", bufs=1)
        r_ps = pp.tile([1, 128], f32, tag="rps", bufs=1)
        v0_ps = pp.tile([1, 512], f32, tag="v0", bufs=1)
        v1_ps = pp.tile([1, 512], f32, tag="v1", bufs=1)

        # ---- PE p-state warm-up: big junk fp32 matmuls (~2048 cycles
        # each) keep the PE continuously busy until the first kw chunk
        # lands, so the real matmuls run at the ramped clock
        for _ in range(4):
            nc.tensor.matmul(warm_ps[:], ones_c128[:], junk[:],
                             start=True, stop=True)

        if stage < 2:
            return

        # ---- r = rw @ xr on the DVE (fp16 accumulate; PE stays on the
        # big streams).  acc_r[p, j] = sum_c rw_sb[p, c*128+j]*xr[128c+p]
        # DVE scalar operands must be fp32 -- cast xrT once on ACT.
        xrT_f = vp.tile([128, 8], f32, tag="xrTf")
        nc.scalar.copy(xrT_f[:], xrT)
        acc_r = vp.tile([128, 128], f16, tag="accr")
        nc.vector.tensor_scalar_mul(acc_r[:], rw_sb[:, 0:128], xrT_f[:, 0:1])
        for c in range(1, 8):
            nc.vector.scalar_tensor_tensor(
                out=acc_r[:], in0=rw_sb[:, c * 128:(c + 1) * 128],
                scalar=xrT_f[:, c:c + 1], in1=acc_r[:],
                op0=Alu.mult, op1=Alu.add)

        # ---- k = kw @ xk: accumulating [1,512] matmuls per d-chunk c
        for c in range(8):
            nc.tensor.matmul(k_ps[:], xkT[:, c:c + 1],
                             kw_sb[:, c * 512:(c + 1) * 512],
                             start=(c == 0), stop=(c == 7))

        # cross-partition reduce of acc_r while the k epilogue drains
        nc.tensor.matmul(r_ps[:], ones16[:], acc_r[:], start=True, stop=True)

        # ---- k epilogue: relu halves in parallel on DVE+ACT, 4 tiny PE
        # transposes, then square+cast to the fp16 stationary columns
        krelu = vp.tile([1, 512], f32, tag="krelu")
        nc.vector.tensor_scalar_max(krelu[:, 0:256], k_ps[:, 0:256], 0.0)
        nc.scalar.activation(krelu[:, 256:512], k_ps[:, 256:512], Act.Relu)
        for m in range(4):
            nc.tensor.transpose(kT_ps[:, m:m + 1],
                                krelu[0:1, m * 128:(m + 1) * 128],
                                ident[0:1, 0:1])
        # keep the PE busy through the epilogue so the vw train doesn't
        # drop out of the ramped p-state
        nc.tensor.matmul(warm_ps[:, 0:256], ones_c128[:], junk[:, 0:256],
                         start=True, stop=True)
        kT_f = vp.tile([128, 4], f32, tag="kTf")
        nc.scalar.copy(kT_f[:], kT_ps[:])
        kT_h = vp.tile([128, 4], f16, tag="kTh")
        nc.scalar.square(kT_h[:], kT_f[:])

        # r output leaves via the SP ring: SP is idle once the bulk
        # weight issues drain, so the 0.6us DMA issue never contends
        # with ACT's critical k-epilogue ops
        r_row = vp.tile([1, 128], f32, tag="r")
        nc.scalar.copy(r_row[:], r_ps[:])
        nc.sync.dma_start(out=r_d[:], in_=r_row[:])

        if stage < 3:
            return

        # ---- v partial: two [1,512] banks (d-halves), accumulated over
        #      4 f-chunks c; v0's last matmul lands first so its DVE
        #      evacuation overlaps v1's last matmul
        for c in range(4):
            nc.tensor.matmul(v0_ps[:], kT_h[:, c:c + 1],
                             vw_sb[:, c * 1024: c * 1024 + 512],
                             start=(c == 0), stop=(c == 3))
            nc.tensor.matmul(v1_ps[:], kT_h[:, c:c + 1],
                             vw_sb[:, c * 1024 + 512: c * 1024 + 1024],
                             start=(c == 0), stop=(c == 3))

        v_sb = vp.tile([1, 1024], f32, tag="vsb")
        nc.vector.tensor_copy(v_sb[:, 0:512], v0_ps[:])
        nc.scalar.copy(v_sb[:, 512:1024], v1_ps[:])
        nc.sync.dma_start(out=v_d[:], in_=v_sb[:])


def _build(stage=3):
    import concourse.bacc as bacc
    import concourse.tile as tile
    from concourse import mybir

    nc = bacc.Bacc("TRN2", target_bir_lowering=False, debug=False,
                   num_devices=N_CORES)
    with tile.TileContext(nc) as tc:
        _body(nc, tc, mybir, stage)
    nc.compile()
    return nc


def _prep_shared(kw, vw, rw):
    """Slice + pack weights per core as fp16 in matmul-moving layout."""
    kw_p, vw_p, rw_p = [], [], []
    for i in range(N_CORES):
        # kw shard (512f, 1024d): [f, c, p] -> [p, c*512+f]
        A = kw[i * FSH:(i + 1) * FSH, :].reshape(512, 8, 128)
        kw_p.append(A.transpose(2, 1, 0).reshape(128, 4096)
                    .astype(np.float16))
        # vw shard (1024d, 512f): [d, c, p] -> [p, c*1024+d]
        B = vw[:, i * FSH:(i + 1) * FSH].reshape(1024, 4, 128)
        vw_p.append(B.transpose(2, 1, 0).reshape(128, 4096)
                    .astype(np.float16))
        # rw shard (128dout, 1024d): [j, c, p] -> [p, c*128+j]
        C = rw[i * DSH:(i + 1) * DSH, :].reshape(128, 8, 128)
        rw_p.append(C.transpose(2, 1, 0).reshape(128, 1024)
                    .astype(np.float16))
    return kw_p, vw_p, rw_p


def _prep_smalls(x, state, tmk, tmr, lnw, lnb):
    """Host LN + token mix; returns [128, 16] fp16 (xkT | xrT)."""
    mu = float(x.mean())
    var = float(np.square(x - mu).mean())
    xn = (x - mu) / np.sqrt(var + LN_EPS) * lnw + lnb
    prev = state[0]
    xk = xn * tmk + prev * (1.0 - tmk)
    xr = xn * tmr + prev * (1.0 - tmr)
    sm = np.concatenate([xk.reshape(8, 128).T, xr.reshape(8, 128).T], axis=1)
    return np.ascontiguousarray(sm).astype(np.float16)


def kernel(x, state, time_mix_k, time_mix_r, kw, vw, rw, ln_weight, ln_bias):
    from concourse import bass_utils

    x = np.asarray(x, dtype=np.float32)
    state = np.asarray(state, dtype=np.float32)
    kw = np.asarray(kw, dtype=np.float32)
    vw = np.asarray(vw, dtype=np.float32)
    rw = np.asarray(rw, dtype=np.float32)
    tmk = np.asarray(time_mix_k, dtype=np.float32)
    tmr = np.asarray(time_mix_r, dtype=np.float32)
    lnw = np.asarray(ln_weight, dtype=np.float32)
    lnb = np.asarray(ln_bias, dtype=np.float32)

    if "nc" not in _STATE:
        _STATE["nc"] = _build()
    nc = _STATE["nc"]

    kw_p, vw_p, rw_p = _prep_shared(kw, vw, rw)
    sm = _prep_smalls(x, state, tmk, tmr, lnw, lnb)

    in_maps = [{"kw_p": kw_p[i], "vw_p": vw_p[i], "rw_p": rw_p[i], "smalls": sm}
               for i in range(N_CORES)]

    res = bass_utils.run_bass_kernel_spmd(nc, in_maps, core_ids=list(range(N_CORES)))

    # unshard: v = sum of partials, r = sigmoid(concat of chunks)
    v = np.zeros(D, dtype=np.float64)
    for i in range(N_CORES):
        v += res.results[i]["v_out"].reshape(D).astype(np.float64)
    r_pre = np.concatenate([res.results[i]["r_out"].reshape(DSH)
                            for i in range(N_CORES)]).astype(np.float64)
    r = 1.0 / (1.0 + np.exp(-r_pre))
    out = x + (r * v).astype(np.float32)

    # xn: exact fp32 LN on host (auxiliary state output)
    mu = float(x.mean())
    var = float(np.square(x - mu).mean())
    xn = (x - mu) / np.sqrt(var + LN_EPS) * lnw + lnb
    return np.asarray(out, dtype=np.float32), np.asarray(xn, dtype=np.float32)


# revision 21
# speedup vs baseline: 1.1063x; 1.0402x over previous
"""RWKV ChannelMixer (single-token) on 8 Trainium2 NeuronCores.

Math (reference):
    xn  = LayerNorm(x) * ln_w + ln_b
    xk  = xn*tmk + prev*(1-tmk);  xr = xn*tmr + prev*(1-tmr)
    r   = sigmoid(rw @ xr)                       # (D,)
    k   = relu(kw @ xk)^2                        # (F,)
    out = x + r * (vw @ k)                       # (D,)
    returns (out, xn)

Sharding (8 cores, no collectives):
    kw: F-row-sharded (512 rows/core)  -> local k chunk (512,)
    vw: F-col-sharded (512 cols/core)  -> partial v_i (1024,)
    rw: D-row-sharded (128 rw rows/core) -> pre-sigmoid r chunk (128,)
    Host computes the O(D) LN/token-mix prologue (0.01% of FLOPs) and
    ships the mixed vectors pre-transposed; host unshard: v = sum_i v_i,
    r = sigmoid(concat(r_i)), out = x + r*v, xn from the same LN.

Device: the memory-bound work -- streaming 2.25MB/core of fp16 weights
(rel err ~2e-4) -- runs as accumulating [1,N] Tensor-engine matmuls that
trail the DMA chunk stream (weights are the moving operand at 1 cycle/
row).  rw's mat-vec runs on the otherwise-idle Vector engine so PE goes
straight from kw into vw.  Junk fp32 matmuls up front keep the PE
p-state ramped.  All engine APs keep partition base 0 (BIR verifier
rejects unaligned/strided partition access).
"""

import sys
import numpy as np

for _p in ("/opt/trn_rl_repo", "/root/.axon_site/_ro/trn_rl_repo"):
    if _p not in sys.path:
        sys.path.append(_p)

D = 1024
F = 4096
N_CORES = 8
FSH = F // N_CORES      # 512 kw rows / vw cols per core
DSH = D // N_CORES      # 128 rw rows per core
LN_EPS = 1e-5

_STATE = {}


def _body(nc, tc, mybir, stage):
    f32 = mybir.dt.float32
    f16 = mybir.dt.float16
    Alu = mybir.AluOpType
    Act = mybir.ActivationFunctionType

    # per-chunk DRAM tensors: each bulk DMA reads fully-contiguous DRAM
    kw_ds = [nc.dram_tensor(f"kw_p{j}", [128, b - a], f16,
                            kind="ExternalInput").ap()
             for j, (a, b) in enumerate(KWCH)]
    vw_ds = [nc.dram_tensor(f"vw_p{j}", [128, b - a], f16,
                            kind="ExternalInput").ap()
             for j, (a, b) in enumerate(VWCH)]
    rw_d = nc.dram_tensor("rw_p", [128, 1024], f16, kind="ExternalInput").ap()
    # xkT (cols 0-7) and xrT (cols 8-15): host-mixed vectors, d-chunk major
    sm_d = nc.dram_tensor("smalls", [128, 16], f16, kind="ExternalInput").ap()

    v_d = nc.dram_tensor("v_out", [1, 1024], f32, kind="ExternalOutput").ap()
    r_d = nc.dram_tensor("r_out", [1, 128], f32, kind="ExternalOutput").ap()

    import contextlib
    with contextlib.ExitStack() as ctx:
        wp = ctx.enter_context(tc.tile_pool(name="w", bufs=1))
        vp = ctx.enter_context(tc.tile_pool(name="v", bufs=1))
        pp = ctx.enter_context(tc.tile_pool(name="ps", bufs=1, space="PSUM"))

        # ---- DMA: tiny smalls on the ACT HWDGE ring (parallel with the
        # SP ring), bulk weights on the SP ring in consumption order:
        # rw first (its mat-vec runs on DVE while PE chews kw), then kw,
        # then vw, chunked so compute trails the stream.
        sm_sb = vp.tile([128, 16], f16, tag="sm")
        nc.scalar.dma_start(out=sm_sb[:], in_=sm_d[:])
        xkT = sm_sb[:, 0:8]
        xrT = sm_sb[:, 8:16]

        kw_sb = wp.tile([128, 4096], f16, tag="kw")
        rw_sb = wp.tile([128, 1024], f16, tag="rw")
        vw_sb = wp.tile([128, 4096], f16, tag="vw")
        # 8 bulk DMAs total: stays within the 8 HWDGE semaphore lanes so
        # the issue pipeline never stalls on semaphore recycling
        if stage >= 2:
            nc.sync.dma_start(out=rw_sb[:], in_=rw_d[:])
            for j, (a, b) in enumerate(KWCH):
                nc.sync.dma_start(out=kw_sb[:, a:b], in_=kw_ds[j][:])
            for j, (a, b) in enumerate(VWCH):
                nc.sync.dma_start(out=vw_sb[:, a:b], in_=vw_ds[j][:])

        # ---- constants; dummy activation forces the single ACT table
        # load early (Relu/Copy share every table set)
        ones16 = vp.tile([128, 1], f16, tag="ones16")
        nc.vector.memset(ones16[:], 1.0)
        ones_c128 = vp.tile([128, 1], f32, tag="ones_c128")
        nc.vector.memset(ones_c128[:], 1.0)
        dummy = vp.tile([1, 1], f32, tag="dummy")
        nc.scalar.activation(dummy[:], ones_c128[0:1, 0:1], Act.Relu)
        junk = vp.tile([128, 512], f32, tag="junk")
        nc.vector.memset(junk[:], 1.0)
        from concourse.masks import make_identity
        ident = vp.tile([128, 128], f32, tag="ident")
        make_identity(nc, ident)

        # ---- PSUM tiles
        warm_ps = pp.tile([1, 512], f32, tag="warm", bufs=1)
        k_ps = pp.tile([1, 512], f32, tag="kps", bufs=1)
        kT_ps = pp.tile([128, 4], f32, tag="kT", bufs=1)
        r_ps = pp.tile([1, 128], f32, tag="rps", bufs=1)
        v0_ps = pp.tile([1, 512], f32, tag="v0", bufs=1)
        v1_ps = pp.tile([1, 512], f32, tag="v1", bufs=1)

        # ---- PE p-state warm-up: big junk fp32 matmuls (~2048 cycles
        # each) keep the PE continuously busy until the first kw chunk
        # lands, so the real matmuls run at the ramped clock
        for _ in range(4):
            nc.tensor.matmul(warm_ps[:], ones_c128[:], junk[:],
                             start=True, stop=True)

        if stage < 2:
            return

        # ---- r = rw @ xr on the DVE (fp16 accumulate; PE stays on the
        # big streams).  acc_r[p, j] = sum_c rw_sb[p, c*128+j]*xr[128c+p]
        # DVE scalar operands must be fp32 -- cast xrT once on ACT.
        xrT_f = vp.tile([128, 8], f32, tag="xrTf")
        nc.scalar.copy(xrT_f[:], xrT)
        acc_r = vp.tile([128, 128], f16, tag="accr")
        nc.vector.tensor_scalar_mul(acc_r[:], rw_sb[:, 0:128], xrT_f[:, 0:1])
        for c in range(1, 8):
            nc.vector.scalar_tensor_tensor(
                out=acc_r[:], in0=rw_sb[:, c * 128:(c + 1) * 128],
                scalar=xrT_f[:, c:c + 1], in1=acc_r[:],
                op0=Alu.mult, op1=Alu.add)

        # ---- k = kw @ xk: accumulating [1,512] matmuls per d-chunk c
        for c in range(8):
            nc.tensor.matmul(k_ps[:], xkT[:, c:c + 1],
                             kw_sb[:, c * 512:(c + 1) * 512],
                             start=(c == 0), stop=(c == 7))

        # cross-partition reduce of acc_r while the k epilogue drains
        nc.tensor.matmul(r_ps[:], ones16[:], acc_r[:], start=True, stop=True)

        # ---- k epilogue: relu halves run truly parallel on DVE+ACT
        # (separate tiles -- a shared tile would serialize them through
        # Tile's whole-tile write ordering), 4 tiny PE transposes, then
        # square+cast to the fp16 stationary columns
        krelu_a = vp.tile([1, 256], f32, tag="krelu_a")
        krelu_b = vp.tile([1, 256], f32, tag="krelu_b")
        nc.vector.tensor_scalar_max(krelu_a[:], k_ps[:, 0:256], 0.0)
        nc.scalar.activation(krelu_b[:], k_ps[:, 256:512], Act.Relu)
        for m in range(4):
            src = krelu_a if m < 2 else krelu_b
            nc.tensor.transpose(kT_ps[:, m:m + 1],
                                src[0:1, (m % 2) * 128:(m % 2 + 1) * 128],
                                ident[0:1, 0:1])
        # keep the PE busy through the epilogue so the vw train doesn't
        # drop out of the ramped p-state
        nc.tensor.matmul(warm_ps[:, 0:256], ones_c128[:], junk[:, 0:256],
                         start=True, stop=True)
        kT_f = vp.tile([128, 4], f32, tag="kTf")
        nc.scalar.copy(kT_f[:], kT_ps[:])
        kT_h = vp.tile([128, 4], f16, tag="kTh")
        nc.scalar.square(kT_h[:], kT_f[:])

        # r output leaves via the SP ring: SP is idle once the bulk
        # weight issues drain, so the 0.6us DMA issue never contends
        # with ACT's critical k-epilogue ops
        r_row = vp.tile([1, 128], f32, tag="r")
        nc.scalar.copy(r_row[:], r_ps[:])
        nc.sync.dma_start(out=r_d[:], in_=r_row[:])

        if stage < 3:
            return

        # ---- v partial: two [1,512] banks (d-halves), accumulated over
        #      4 f-chunks c; v0's last matmul lands first so its DVE
        #      evacuation overlaps v1's last matmul
        for c in range(4):
            nc.tensor.matmul(v0_ps[:], kT_h[:, c:c + 1],
                             vw_sb[:, c * 1024: c * 1024 + 512],
                             start=(c == 0), stop=(c == 3))
            nc.tensor.matmul(v1_ps[:], kT_h[:, c:c + 1],
                             vw_sb[:, c * 1024 + 512: c * 1024 + 1024],
                             start=(c == 0), stop=(c == 3))

        v_sb = vp.tile([1, 1024], f32, tag="vsb")
        nc.vector.tensor_copy(v_sb[:, 0:512], v0_ps[:])
        nc.scalar.copy(v_sb[:, 512:1024], v1_ps[:])
        nc.sync.dma_start(out=v_d[:], in_=v_sb[:])


def _build(stage=3):
    import concourse.bacc as bacc
    import concourse.tile as tile
    from concourse import mybir

    nc = bacc.Bacc("TRN2", target_bir_lowering=False, debug=False,
                   num_devices=N_CORES)
    with tile.TileContext(nc) as tc:
        _body(nc, tc, mybir, stage)
    nc.compile()
    return nc


KWCH = [(0, 2048), (2048, 3584), (3584, 4096)]
VWCH = [(0, 2048), (2048, 3584), (3584, 3840), (3840, 4096)]


def _prep_shared(kw, vw, rw):
    """Slice + pack weights per core as fp16 in matmul-moving layout.

    Returns per-core dicts of dram-tensor name -> array; the big mats are
    split into per-chunk tensors so each DMA reads contiguous DRAM.
    """
    maps = []
    for i in range(N_CORES):
        # kw shard (512f, 1024d): [f, c, p] -> [p, c*512+f]
        A = (kw[i * FSH:(i + 1) * FSH, :].reshape(512, 8, 128)
             .transpose(2, 1, 0).reshape(128, 4096).astype(np.float16))
        # vw shard (1024d, 512f): [d, c, p] -> [p, c*1024+d]
        B = (vw[:, i * FSH:(i + 1) * FSH].reshape(1024, 4, 128)
             .transpose(2, 1, 0).reshape(128, 4096).astype(np.float16))
        # rw shard (128dout, 1024d): [j, c, p] -> [p, c*128+j]
        C = (rw[i * DSH:(i + 1) * DSH, :].reshape(128, 8, 128)
             .transpose(2, 1, 0).reshape(128, 1024).astype(np.float16))
        m = {"rw_p": C}
        for j, (a, b) in enumerate(KWCH):
            m[f"kw_p{j}"] = np.ascontiguousarray(A[:, a:b])
        for j, (a, b) in enumerate(VWCH):
            m[f"vw_p{j}"] = np.ascontiguousarray(B[:, a:b])
        maps.append(m)
    return maps


def _prep_smalls(x, state, tmk, tmr, lnw, lnb):
    """Host LN + token mix; returns [128, 16] fp16 (xkT | xrT)."""
    mu = float(x.mean())
    var = float(np.square(x - mu).mean())
    xn = (x - mu) / np.sqrt(var + LN_EPS) * lnw + lnb
    prev = state[0]
    xk = xn * tmk + prev * (1.0 - tmk)
    xr = xn * tmr + prev * (1.0 - tmr)
    sm = np.concatenate([xk.reshape(8, 128).T, xr.reshape(8, 128).T], axis=1)
    return np.ascontiguousarray(sm).astype(np.float16)


def kernel(x, state, time_mix_k, time_mix_r, kw, vw, rw, ln_weight, ln_bias):
    from concourse import bass_utils

    x = np.asarray(x, dtype=np.float32)
    state = np.asarray(state, dtype=np.float32)
    kw = np.asarray(kw, dtype=np.float32)
    vw = np.asarray(vw, dtype=np.float32)
    rw = np.asarray(rw, dtype=np.float32)
    tmk = np.asarray(time_mix_k, dtype=np.float32)
    tmr = np.asarray(time_mix_r, dtype=np.float32)
    lnw = np.asarray(ln_weight, dtype=np.float32)
    lnb = np.asarray(ln_bias, dtype=np.float32)

    if "nc" not in _STATE:
        _STATE["nc"] = _build()
    nc = _STATE["nc"]

    maps = _prep_shared(kw, vw, rw)
    sm = _prep_smalls(x, state, tmk, tmr, lnw, lnb)

    in_maps = [dict(maps[i], smalls=sm) for i in range(N_CORES)]

    res = bass_utils.run_bass_kernel_spmd(nc, in_maps, core_ids=list(range(N_CORES)))

    # unshard: v = sum of partials, r = sigmoid(concat of chunks)
    v = np.zeros(D, dtype=np.float64)
    for i in range(N_CORES):
        v += res.results[i]["v_out"].reshape(D).astype(np.float64)
    r_pre = np.concatenate([res.results[i]["r_out"].reshape(DSH)
                            for i in range(N_CORES)]).astype(np.float64)
    r = 1.0 / (1.0 + np.exp(-r_pre))
    out = x + (r * v).astype(np.float32)

    # xn: exact fp32 LN on host (auxiliary state output)
    mu = float(x.mean())
    var = float(np.square(x - mu).mean())
    xn = (x - mu) / np.sqrt(var + LN_EPS) * lnw + lnb
    return np.asarray(out, dtype=np.float32), np.asarray(xn, dtype=np.float32)
